# revision 4
# baseline (speedup 1.0000x reference)
"""MLA attention kernel v2 for TRN2, SPMD over 8 NeuronCores.

Sharding: core c = 4*b + g  (b = batch 0..1, g = head-group 0..3, 4 heads each).
Per-t software pipeline per head-pair: QK (row-split pair) -> exp (ACT) ->
PV (col-split pair) -> L (4-way col-tiled ones-matmul), with proj/wo work
woven in as fillers so the PE queue never blocks on ACT.

Layouts per core (b, g):
    qT = (Wq_g*scale)^T x^T + bq_g*scale     [256, 2048]  bf16  (2 pairs x 128)
    latT = Wl^T x^T + bl                     [256, 2048]  bf16
    kT = Wk_g^T latT                         [256, 2048]  bf16
    v  = latT^T Wv_g                         [2048, 256]  bf16
    pair m, key-chunk t: S[:,j,:] = kT_h^T qT_h (h=2m+j) -> exp -> pt
    ot[64j:64j+64,:] += v_h^T pt_j ; L[32h] += ones^T pt_j
    at = ot * bc(1/L) ; partial = At Wo_g    [2048, 1024]  bf16 out
Host sums the 4 partials per batch (f32) and adds (bv @ Wo + bo).
"""
import contextlib
import ctypes
import os
import sys
import types

if "/opt/trn_rl_repo" not in sys.path:
    sys.path.insert(0, "/opt/trn_rl_repo")

import numpy as np
import ml_dtypes

NPBF16 = ml_dtypes.bfloat16
NPFP8 = ml_dtypes.float8_e4m3
SCALE = 64 ** -0.5
_STATE = {}


# ---------------------------------------------------------------- ntff shim
def _install_ntff_shim():
    if "antenv.axon_hooks" in sys.modules:
        return
    try:
        import antenv
    except ImportError:
        return

    so_path = "/opt/axon/libaxon_pjrt.so"

    def _hook_factory():
        try:
            lib = ctypes.CDLL(so_path)
        except OSError:
            return None
        if not hasattr(lib, "axon_start_nrt_profile"):
            return None
        lib.axon_start_nrt_profile.argtypes = [ctypes.POINTER(ctypes.c_int64), ctypes.c_size_t]
        lib.axon_start_nrt_profile.restype = ctypes.c_int64
        lib.axon_stop_nrt_profile.argtypes = [ctypes.c_char_p]
        lib.axon_stop_nrt_profile.restype = ctypes.c_int64

        @contextlib.contextmanager
        def _hook(output_dir, device_ids):
            import jax

            jax.devices()
            if device_ids:
                ids = (ctypes.c_int64 * len(device_ids))(*device_ids)
                rc = lib.axon_start_nrt_profile(ids, len(device_ids))
            else:
                rc = lib.axon_start_nrt_profile(None, 0)
            if rc != 0:
                raise RuntimeError(f"axon_start_nrt_profile rc={rc}")
            try:
                yield
            finally:
                n = lib.axon_stop_nrt_profile(str(output_dir).encode())
                print(f"profile: {n} file(s) written to {output_dir}", file=sys.stderr)

        return _hook

    import antenv

    mod = types.ModuleType("antenv.axon_hooks")
    _state = {"hook": _hook_factory()}
    mod.set_axon_ntff_profile_hook = lambda h: _state.__setitem__("hook", h)
    mod.get_axon_ntff_profile_hook = lambda: _state["hook"]
    sys.modules["antenv.axon_hooks"] = mod
    antenv.axon_hooks = mod


# ---------------------------------------------------------------- bass build
def _build_nc():
    import concourse.bass as bass  # noqa: F401
    import concourse.tile as tile
    from concourse import bacc, mybir

    BF16 = mybir.dt.bfloat16
    FP8 = mybir.dt.float8e4
    F32 = mybir.dt.float32
    EXP = mybir.ActivationFunctionType.Exp

    nc = bacc.Bacc(None, target_bir_lowering=False, debug=False)

    xT = nc.dram_tensor("xT", [128, 4, 8, 512], BF16, kind="ExternalInput")
    wq = nc.dram_tensor("wq", [128, 8, 256], BF16, kind="ExternalInput")
    bq = nc.dram_tensor("bq", [128, 2], F32, kind="ExternalInput")
    wl = nc.dram_tensor("wl", [128, 8, 256], BF16, kind="ExternalInput")
    bl = nc.dram_tensor("bl", [128, 2], F32, kind="ExternalInput")
    wk = nc.dram_tensor("wk", [128, 2, 256], BF16, kind="ExternalInput")
    wv = nc.dram_tensor("wv", [128, 2, 256], BF16, kind="ExternalInput")
    wo = nc.dram_tensor("wo", [128, 2, 1024], BF16, kind="ExternalInput")
    # packed output: (ic, u, row, col) -> each [128,1024] store is one
    # contiguous dram block (2KB per partition); host reassembles
    out = nc.dram_tensor("out", [4, 4, 128, 1024], BF16, kind="ExternalOutput")

    with nc.allow_low_precision("bf16 intermediates by design"), tile.TileContext(nc) as tc:
        with (
            tc.tile_pool(name="wpool", bufs=1) as wpool,
            tc.tile_pool(name="xpool", bufs=1) as xpool,
            tc.tile_pool(name="proj", bufs=1) as proj,
            tc.tile_pool(name="ptp", bufs=22) as ptp,
            tc.tile_pool(name="atp", bufs=4) as atp,
            tc.tile_pool(name="obp", bufs=2) as obp,
            tc.tile_pool(name="rpool", bufs=2) as rpool,
            tc.tile_pool(name="ps", bufs=2, space="PSUM") as ps,
        ):
            # ---------------- SBUF constants + inputs
            x_sb = xpool.tile([128, 4, 8, 512], BF16)
            wq_sb = wpool.tile([128, 8, 256], BF16)
            wl_sb = wpool.tile([128, 8, 256], BF16)
            wk_sb = wpool.tile([128, 2, 256], BF16)
            wv_sb = wpool.tile([128, 2, 256], BF16)
            wo_sb = wpool.tile([128, 2, 1024], BF16)
            bq_sb = wpool.tile([128, 2], F32)
            bl_sb = wpool.tile([128, 2], F32)
            ones_sb = wpool.tile([128, 1], BF16)
            ones_k1 = wpool.tile([128, 64], BF16)
            warm_sb = wpool.tile([128, 512], BF16)
            nc.vector.memset(warm_sb[:], 0.25)
            nc.vector.memset(ones_sb[:], 1.0)
            nc.vector.memset(ones_k1[:], 1.0)

            # x quarters on the ACT-engine DGE ring, weights on sync;
            # lat(0)/q(0) start as soon as quarter 0 lands
            nc.sync.dma_start(out=wl_sb[:], in_=wl[:])
            nc.sync.dma_start(out=bl_sb[:], in_=bl[:])
            nc.scalar.dma_start(out=x_sb[:, 0], in_=xT[:, 0])
            nc.sync.dma_start(out=wq_sb[:], in_=wq[:])
            nc.sync.dma_start(out=bq_sb[:], in_=bq[:])
            nc.scalar.dma_start(out=x_sb[:, 1], in_=xT[:, 1])
            nc.sync.dma_start(out=wk_sb[:], in_=wk[:])
            nc.sync.dma_start(out=wv_sb[:], in_=wv[:])
            nc.scalar.dma_start(out=x_sb[:, 2], in_=xT[:, 2])
            nc.scalar.dma_start(out=x_sb[:, 3], in_=xT[:, 3])
            nc.sync.dma_start(out=wo_sb[:], in_=wo[:])

            latT_n = [proj.tile([128, 2, 512], BF16, name=f"latT_{i}") for i in range(4)]
            qT_n = [proj.tile([128, 2, 512], BF16, name=f"qT_{i}") for i in range(4)]
            kT_n = [proj.tile([128, 2, 512], BF16, name=f"kT_{i}") for i in range(4)]
            v_sb = proj.tile([128, 16, 256], BF16)

            # ---------------- PSUM helpers (8 banks: S 4, ot 2, L 1, misc 1)
            def s_ps(name):
                return ps.tile([128, 2, 512], F32, tag="s", name=name, bufs=2)

            def ot_ps(name):
                return ps.tile([128, 512], F32, tag="ot", name=name, bufs=2)

            def l_ps(name):
                return ps.tile([128, 512], F32, tag="L", name=name, bufs=1)

            def misc_ps(name):
                return ps.tile([128, 512], F32, tag="m", name=name, bufs=1)

            # ---------------- HAM warm-up spanning the input DMA wait
            warm_ps = misc_ps("warm_ps")
            for i in range(26):
                nc.tensor.matmul(
                    warm_ps[:], warm_sb[:, 0:128], warm_sb[:],
                    start=(i == 0), stop=(i == 25),
                )

            # ---------------- projection emitters (chains -> misc bank)
            def emit_lat(n, m):
                acc = misc_ps(f"lat_ps_{m}_{n}")
                for k in range(8):
                    nc.tensor.matmul(
                        acc[:], wl_sb[:, k, 128 * m : 128 * m + 128], x_sb[:, n, k, :],
                        start=(k == 0), stop=(k == 7),
                    )
                nc.vector.tensor_scalar_add(
                    out=latT_n[n][:, m, :], in0=acc[:], scalar1=bl_sb[:, m : m + 1]
                )

            def emit_kt(n, m):
                acc = misc_ps(f"kt_ps_{m}_{n}")
                for k in range(2):
                    nc.tensor.matmul(
                        acc[:], wk_sb[:, k, 128 * m : 128 * m + 128], latT_n[n][:, k, :],
                        start=(k == 0), stop=(k == 1),
                    )
                nc.vector.tensor_copy(out=kT_n[n][:, m, :], in_=acc[:])

            def emit_v(t0, nt=2):
                acc = misc_ps(f"v_ps_{t0}")
                for tt in range(nt):
                    t = t0 + tt
                    for k in range(2):
                        nc.tensor.matmul(
                            acc[:, 256 * tt : 256 * tt + 256],
                            latT_n[t // 4][:, k, 128 * (t % 4) : 128 * (t % 4) + 128],
                            wv_sb[:, k, :],
                            start=(k == 0), stop=(k == 1), skip_group_check=True,
                        )
                for tt in range(nt):
                    nc.vector.tensor_copy(
                        out=v_sb[:, t0 + tt, :], in_=acc[:, 256 * tt : 256 * tt + 256]
                    )

            def emit_q(ic, m):
                acc = misc_ps(f"q_ps_{m}_{ic}")
                for k in range(8):
                    nc.tensor.matmul(
                        acc[:], wq_sb[:, k, 128 * m : 128 * m + 128], x_sb[:, ic, k, :],
                        start=(k == 0), stop=(k == 7),
                    )
                nc.vector.tensor_scalar_add(
                    out=qT_n[ic][:, m, :], in0=acc[:], scalar1=bq_sb[:, m : m + 1]
                )

            # ---------------- norm + output emitters
            recf = {}  # (ic) -> f32 scratch
            recb = {}  # (ic) -> bf16 recip
            bcs = {}   # (ic, m) -> bc sbuf f32
            ats = {}   # (ic, m) -> at bf16

            def emit_recip(ic, L):
                rf = rpool.tile([128, 512], F32, tag="rf", name=f"rf_{ic}")
                rb = rpool.tile([128, 512], BF16, tag="rb", name=f"rb_{ic}")
                nc.vector.reciprocal_approx_fast(out=rf[:], in_=L[:])
                nc.vector.tensor_copy(out=rb[:], in_=rf[:])
                recb[ic] = rb

            def emit_bc(ic, m):
                rb = recb[ic]
                bc_ps = misc_ps(f"bcp_{ic}_{m}")
                for j in range(2):
                    h = 2 * m + j
                    r = 32 * h
                    nc.tensor.matmul(
                        bc_ps[64 * j : 64 * j + 64, :],
                        ones_k1[r : r + 1, 0:64],
                        rb[r : r + 1, :],
                        start=True, stop=True,
                        tile_position=(r, 64 * j), skip_group_check=True,
                    )
                bc = rpool.tile([128, 512], F32, tag="bc", name=f"bc_{ic}_{m}")
                nc.vector.tensor_copy(out=bc[:], in_=bc_ps[:])
                bcs[(ic, m)] = bc

            def emit_at(ic, m, ot):
                at = atp.tile([128, 512], BF16, tag="at", name=f"at_{ic}_{m}")
                nc.vector.tensor_mul(out=at[:], in0=ot[:], in1=bcs[(ic, m)][:])
                ats[(ic, m)] = at

            obs = {}

            def emit_wo(ic, u, n2, alt=False):
                if alt:
                    # tail: S banks are free; alternate with misc bank so
                    # consecutive chains don't serialize on the DVE evac
                    st = s_ps(f"wos_{ic}_{u}_{n2}")
                    acc = st[:, 0, :]
                else:
                    acc = misc_ps(f"wo_{ic}_{u}_{n2}")
                for m in range(2):
                    nc.tensor.matmul(
                        acc[:],
                        ats[(ic, m)][:, 128 * u : 128 * u + 128],
                        wo_sb[:, m, 512 * n2 : 512 * n2 + 512],
                        start=(m == 0), stop=(m == 1),
                    )
                if (ic, u) not in obs:
                    obs[(ic, u)] = obp.tile([128, 1024], BF16, tag="ob", name=f"ob_{ic}_{u}")
                ob = obs[(ic, u)]
                nc.vector.tensor_copy(out=ob[:, 512 * n2 : 512 * n2 + 512], in_=acc[:])
                if n2 == 1:
                    nc.sync.dma_start(out=out[ic, u], in_=ob[:])

            # ---------------- filler queue
            fillers = []

            def pump(n=1):
                for _ in range(n):
                    if fillers:
                        fillers.pop(0)()

            def drain_fillers():
                while fillers:
                    fillers.pop(0)()

            # ---------------- attention pair pipeline
            Ls = {}

            def emit_attn_pair(p):
                ic, m = divmod(p, 2)
                qTc = qT_n[ic]
                pts = {}
                ot = ot_ps(f"ot_{p}")

                def qk(t):
                    S = s_ps(f"S_{p}_{t}")
                    kTc = kT_n[t // 4]
                    ksl = slice(128 * (t % 4), 128 * (t % 4) + 128)
                    # Sacrificial 1x1 matmul: absorbs the WAR wait on the
                    # recycled S banks (ACT exp read) so the real QK pair
                    # issues waitless and the scheduler keeps it adjacent.
                    # Only even pairs (ACT-paced) need it; pair 0 and the
                    # odd pairs (PE-bound: L sums + drains) never wait.
                    if p > 0 and p % 2 == 0:
                        nc.tensor.matmul(
                            S[0:1, 0, 0:1], ones_sb[0:1, 0:1], ones_sb[0:1, 0:1],
                            start=True, stop=True, skip_group_check=True,
                        )
                    nc.tensor.matmul(
                        S[:, 0, :], kTc[0:64, m, ksl], qTc[0:64, m, :],
                        start=True, stop=True,
                    )
                    nc.tensor.matmul(
                        S[:, 1, :], kTc[64:128, m, ksl], qTc[64:128, m, :],
                        start=True, stop=True,
                    )
                    pt = ptp.tile([128, 2, 512], BF16, tag="pt", name=f"pt_{p}_{t}")
                    nc.scalar.activation(pt[:], S[:], EXP)
                    pts[t] = pt

                def pv(t):
                    pt = pts[t]
                    for j in range(2):
                        h = 2 * m + j
                        nc.tensor.matmul(
                            ot[64 * j : 64 * j + 64, :],
                            v_sb[:, t, 64 * h : 64 * h + 64],
                            pt[:, j, :],
                            start=(t == 0), stop=(t == 15), skip_group_check=True,
                        )

                def lsum(t):
                    if m == 0:
                        return
                    L = Ls.get(ic)
                    if L is None:
                        L = l_ps(f"L_{ic}")
                        Ls[ic] = L
                    prev = pt_prev[ic]
                    for pm, ptt in ((0, prev[t]), (1, pts[t])):
                        for j in range(2):
                            h = 2 * pm + j
                            nc.tensor.matmul(
                                L[32 * h : 32 * h + 1, :],
                                ones_sb[:],
                                ptt[:, j, :],
                                start=(t == 0), stop=(t == 15),
                                tile_position=(0, 32 * h), skip_group_check=True,
                            )

                # 2-t batches: 4 same-shape QK MMs, then 4 PV, then 8 L —
                # homogeneous runs pipeline ~2x better than mixed singles.
                # PV lags 6 chunks so the ic-boundary norm (ot recycle) is
                # off the critical path; the pair's drain is handed to the
                # NEXT pair's filler queue so pair boundaries overlap.
                for tt in range(0, 16, 2):
                    qk(tt)
                    qk(tt + 1)
                    if tt >= 6:
                        pv(tt - 6)
                        pv(tt - 5)
                    if tt >= 4:
                        lsum(tt - 4)
                        lsum(tt - 3)
                    pump(2)
                drains = [
                    lambda: (lsum(12), lsum(13), pv(10), pv(11)),
                    lambda: (lsum(14), lsum(15), pv(12), pv(13)),
                    lambda: (pv(14), pv(15)),
                ]
                if m == 0:
                    pt_prev[ic] = pts
                return ot, drains

            pt_prev = {}
            ots = {}

            # ---------------- static schedule
            # Head: lat(0), kT(0), q(0), v(0,1) before pair 0.
            emit_lat(0, 0)
            emit_lat(0, 1)
            emit_kt(0, 0)
            emit_kt(0, 1)
            emit_q(0, 0)
            emit_q(0, 1)
            emit_v(0)

            # fillers for pair (0,0), one per t-cycle starting at t=2.
            # deps: kT(n)/lat(n) before QK(t=4n); v chunk t before PV at t+2.
            fillers += [
                lambda: (emit_lat(1, 0), emit_lat(1, 1)),          # t=2
                lambda: (emit_kt(1, 0), emit_kt(1, 1), emit_v(2)), # t=3
                lambda: emit_v(4),                                  # t=4
                lambda: (emit_lat(2, 0), emit_lat(2, 1)),          # t=5
                lambda: (emit_kt(2, 0), emit_kt(2, 1)),            # t=6
                lambda: emit_v(6),                                  # t=7
                lambda: emit_v(8),                                  # t=8
                lambda: (emit_lat(3, 0), emit_lat(3, 1)),          # t=9
                lambda: (emit_kt(3, 0), emit_kt(3, 1)),            # t=10
            ]
            ots[(0, 0)], d_even = emit_attn_pair(0)
            drain_fillers()
            # v(10..15) feed only the pv drains (executed in pair (0,1)),
            # so emit them here, before the drains, to lighten pair 0
            fillers += [
                lambda: emit_v(10),
                lambda: emit_v(12),
                lambda: emit_v(14),
            ]
            fillers += d_even
            fillers += [
                lambda: emit_q(1, 0),
                lambda: emit_q(1, 1),
            ]
            ots[(0, 1)], d_odd = emit_attn_pair(1)
            drain_fillers()

            def norm_sched(ic):
                # after pair (ic,1): recip -> bc -> at for both pairs
                return [
                    lambda: emit_recip(ic, Ls.pop(ic)),
                    lambda: emit_bc(ic, 0),
                    lambda: emit_at(ic, 0, ots[(ic, 0)]),
                    lambda: emit_bc(ic, 1),
                    lambda: emit_at(ic, 1, ots[(ic, 1)]),
                ]

            def wo_sched(ic):
                return [
                    (lambda u=u, n2=n2: emit_wo(ic, u, n2))
                    for u in range(4) for n2 in range(2)
                ]

            for ic in range(1, 4):
                fillers += d_odd
                fillers += norm_sched(ic - 1)
                fillers += wo_sched(ic - 1)
                ots[(ic, 0)], d_even = emit_attn_pair(2 * ic)
                drain_fillers()
                fillers += d_even
                if ic < 3:
                    # q(ic+1) must be emitted before pair(ic+1,0) reads it
                    fillers += [
                        lambda ic=ic: emit_q(ic + 1, 0),
                        lambda ic=ic: emit_q(ic + 1, 1),
                    ]
                ots[(ic, 1)], d_odd = emit_attn_pair(2 * ic + 1)
                drain_fillers()

            # tail: drain last pair (L first so recip overlaps the PV
            # drain), norm + wo for ic=3 with keep-warm matmuls so HAM
            # stays at full clock through the sparse tail
            d_odd[0](); d_odd[1](); d_odd[2]()

            def keep_warm(n):
                wt = s_ps(f"tw_{keep_warm.i}")
                keep_warm.i += 1
                for j in range(n):
                    nc.tensor.matmul(
                        wt[:, 0, :], warm_sb[:, 0:128], warm_sb[:],
                        start=(j == 0), stop=(j == n - 1),
                    )
            keep_warm.i = 0

            norm_items = norm_sched(3)
            norm_items[0]()
            keep_warm(3)
            for f in norm_items[1:]:
                f()
            for i, (u, n2) in enumerate([(u, n2) for u in range(4) for n2 in range(2)]):
                emit_wo(3, u, n2, alt=(i % 2 == 1))
                if i in (1, 4):
                    keep_warm(2)

    nc.compile()
    return nc


def _get_nc():
    if "nc" not in _STATE:
        _STATE["nc"] = _build_nc()
    return _STATE["nc"]


# ---------------------------------------------------------------- host side
def _pack_k(a, kchunks, dt=NPBF16):
    """[K, N] f32 -> [128, kchunks, N] (K = 128*kchunks)."""
    K, N = a.shape
    return np.ascontiguousarray(
        np.asarray(a, np.float32).reshape(kchunks, 128, N).transpose(1, 0, 2)
    ).astype(dt)


def kernel(x, Wq, bq, Wl, bl, Wk, bk, Wv, bv, Wo, bo):
    x = np.asarray(x, np.float32)
    Wq = np.asarray(Wq, np.float32)
    bq = np.asarray(bq, np.float32)
    Wl = np.asarray(Wl, np.float32)
    bl = np.asarray(bl, np.float32)
    Wk = np.asarray(Wk, np.float32)
    Wv = np.asarray(Wv, np.float32)
    bv = np.asarray(bv, np.float32)
    Wo = np.asarray(Wo, np.float32)
    bo = np.asarray(bo, np.float32)

    from concourse.bass_utils import run_bass_kernel_spmd

    trace = os.environ.get("KERNEL_TRACE", "0") == "1"
    if trace:
        _install_ntff_shim()

    wl_p = _pack_k(Wl, 8)
    bl_p = np.ascontiguousarray(bl.reshape(2, 128).T).astype(np.float32)
    in_maps = []
    for c in range(8):
        b, g = divmod(c, 4)
        sl = slice(256 * g, 256 * g + 256)
        xp = _pack_k(x[b].T, 8)  # [128, 8, 2048]
        xp2 = np.ascontiguousarray(
            xp.reshape(128, 8, 4, 512).transpose(0, 2, 1, 3)
        )  # [128, 4, 8, 512], contiguous per partition per quarter
        in_maps.append(
            {
                "xT": xp2,
                "wq": _pack_k(Wq[:, sl] * SCALE, 8),
                "bq": np.ascontiguousarray((bq[sl] * SCALE).reshape(2, 128).T).astype(np.float32),
                "wl": wl_p,
                "bl": bl_p,
                "wk": _pack_k(Wk[:, sl], 2),
                "wv": _pack_k(Wv[:, sl], 2),
                "wo": _pack_k(Wo[sl, :], 2),
            }
        )

    nc = _get_nc()
    tdir = os.environ.get("KERNEL_TRACE_TMPDIR") or None
    res = run_bass_kernel_spmd(nc, in_maps, core_ids=list(range(8)), trace=trace, tmpdir=tdir)
    if trace and res.exec_time_ns is not None:
        print(f"HW exec time: {res.exec_time_ns} ns")
        _STATE["exec_time_ns"] = res.exec_time_ns

    def unpack(o):
        # [4, 4, 128, 1024] (ic, u, r, c) -> [2048, 1024]
        return np.asarray(o, np.float32).reshape(2048, 1024)

    parts = [unpack(res.results[c]["out"]) for c in range(8)]
    const = (bv @ Wo + bo).astype(np.float32)
    out = np.empty((2, 2048, 1024), np.float32)
    for b in range(2):
        out[b] = parts[4 * b] + parts[4 * b + 1] + parts[4 * b + 2] + parts[4 * b + 3] + const
    return out
